# revision 1
# baseline (speedup 1.0000x reference)
"""Trainium2 Bass kernel for the neural-ODE Euler integration problem.

Model (per batch element b, 999 sequential steps):
    all_init = [x0, z0]                                   (16)
    a1 = [all_init, xz - all_init, xz] @ W1 + b1          (48 -> 256)
       = all_init @ (W1a - W1b) + x @ Ux + z_eff @ Uz + b1
    h1 = elu(a1); a2 = h1 @ W2 + b2; h2 = elu(a2)
    a3 = h2 @ W3 + b3; h3 = elu(a3); dx = h3 @ W4 + b4
    x <- x + dt * dx
z_eff switches from z[:, i] to z_jump once t >= event_t.

Kernel strategy: pure data parallel, 128 trajectories per core (8 cores).
Feature-major layout (features on partitions, batch on the free dim).
Time steps are packed 16-per-partition-group into big SBUF-resident
streams [128, 63*128].  elu is computed as  elu(a)+1 = relu(a) + min(exp(a),1)
with the -1 absorbed into the next layer's bias; relu/min streams feed the
next matmul as two accumulating rhs streams.  The z_eff stream is consumed
directly from its packed layout by K=64 matmuls whose lhsT is zero except
for the 8 rows matching the step's slot.  fp16 matmuls, fp32 PSUM/state.
"""

import numpy as np
import sys

if '/opt/trn_rl_repo' not in sys.path:
    sys.path.insert(0, '/opt/trn_rl_repo')

import concourse.bass as bass
import concourse.bacc as bacc
import concourse.mybir as mybir
from concourse.tile import TileContext
from concourse import bass_utils

F32 = mybir.dt.float32
F16 = mybir.dt.float16
AF = mybir.ActivationFunctionType
OP = mybir.AluOpType

B, T, XD, ZD, HID = 1024, 1000, 8, 8, 256
NCORES = 8
PB = B // NCORES          # batch per core = 128
SPG = 16                  # steps packed per partition group
NG = 63                   # groups -> 1008 slots >= 1000
NSLOT = NG * SPG
NSTEPS = T - 1            # 999 Euler steps

LAST_RESULTS = None       # set by kernel(): BassKernelResults


def _build(nsteps, ng):
    """Build + compile the Bass program (same program for all 8 cores)."""
    nc = bacc.Bacc("TRN2", target_bir_lowering=False, debug=False,
                   num_devices=NCORES)

    ncols = ng * PB
    nslot = ng * SPG

    # ---- DRAM I/O ----
    d = {}
    def din(name, shape, dt):
        d[name] = nc.dram_tensor(name, shape, dt, kind="ExternalInput").ap()
    din("zpack", [128, ncols], F32)     # z stream, packed, zero-padded
    din("etb",   [128, PB], F32)        # event_t broadcast across partitions
    din("zjb",   [128, PB], F16)        # z_jump tiled over the 16 step-rows
    din("aiext", [18, PB], F16)         # [all_init (16); ones (2)]
    din("x0f",   [8, PB], F32)          # x0 fp32 (Euler in1 for step 0)
    din("x0h",   [8, PB], F16)          # x0 fp16 (stage 0)
    din("tvec",  [128, ng], F32)        # t_i replicated x8 along partitions
    din("dtvec", [8, nslot], F32)       # dt_i on partitions 0-7
    din("vux",   [128, 256], F16)       # [Ux (8); V (16); b1_hi; b1_lo; 0-pad]
    din("uzv",   [128, 16 * 256], F16)  # 16 variants: Uz at rows 8r, 0-pad
    din("w2p",   [128, 512], F16)       # W2[kc*128+p, h*128+m] at col (kc*2+h)*128+m
    din("w3p",   [128, 512], F16)
    din("w4p",   [128, 16], F16)        # W4[kc*128+p, m] at col kc*8+m
    din("b2e",   [128, 256], F16)       # hi/lo of b2 - colsum(W2), 0-pad
    din("b3e",   [128, 256], F16)
    din("b4e",   [2, 8], F16)
    din("ones2", [128, PB], F16)    # rows 0-1 ones, rest zero
    xout_d = nc.dram_tensor("xout_d", [8, nslot + SPG, PB], F32,
                            kind="ExternalOutput").ap()

    with TileContext(nc) as tc:
        with tc.tile_pool(name="const", bufs=1) as cpool, \
             tc.tile_pool(name="work", bufs=3) as wpool, \
             tc.tile_pool(name="psum", bufs=2, space="PSUM") as ppool:

            # ---- static SBUF tiles + initial DMA ----
            sb = {}
            for name in d:
                shape = [int(s) for s in d[name].shape]
                dt = d[name].dtype
                sb[name] = cpool.tile(shape, dt, name=name, tag=name)
                nc.sync.dma_start(out=sb[name][:], in_=d[name])

            zeff = cpool.tile([128, ncols], F16, name="zeff", tag="zeff")

            # three rotating [26, PB] rhs tiles: rows 0-17 = aiext (static),
            # rows 18-25 = x_i fp16 (written per step)
            stages = []
            for k in range(3):
                st = wpool.tile([128, PB], F16, name=f"aistage{k}",
                                tag=f"aistage{k}", bufs=1)
                nc.vector.memset(st[:], 0)
                nc.sync.dma_start(out=st[8:26, :], in_=d["aiext"])
                stages.append(st)

            # ---- prepass: z_eff = (t_i >= event_t) ? z_jump : z ----
            for g in range(ng):
                gc = slice(g * PB, (g + 1) * PB)
                mk = wpool.tile([128, PB], mybir.dt.uint8, name="mask",
                                tag="mask")
                nc.vector.tensor_scalar(
                    out=mk[:], in0=sb["etb"][:],
                    scalar1=sb["tvec"][:, g:g + 1], scalar2=None, op0=OP.is_le)
                nc.vector.tensor_copy(out=zeff[:, gc], in_=sb["zpack"][:, gc])
                nc.vector.copy_predicated(out=zeff[:, gc], mask=mk[:],
                                          data=sb["zjb"][:])

            # ---- main sequential loop ----
            xprev = None        # AP holding x_i fp32 (partitions 0-7)
            rings = {}          # group -> ring tile
            for i in range(nsteps):
                g, s = i // SPG, i % SPG
                gc = slice(g * PB, (g + 1) * PB)

                # stage_i = x_i fp16 at rows 18-25 of the rotating rhs tile
                stage = stages[i % 3]
                if i == 0:
                    nc.sync.dma_start(out=stage[0:8, :], in_=d["x0h"])
                    xprev = cpool.tile([8, PB], F32, name="xcur0", tag="xcur0")
                    nc.sync.dma_start(out=xprev[:], in_=d["x0f"])

                r = s % 16

                # L1: a1 = (all_init@V + b1 + x@Ux) + z_eff@Uz
                a1 = ppool.tile([128, 256], F32, name="a1", tag="a1")
                for h in range(2):
                    hs = slice(h * 128, (h + 1) * 128)
                    nc.tensor.matmul(
                        a1[:, hs],
                        lhsT=sb["uzv"][:, r * 256 + h * 128:r * 256 + (h + 1) * 128],
                        rhs=zeff[:, gc], start=True, stop=False)
                    nc.tensor.matmul(a1[:, hs], lhsT=sb["vux"][:, hs],
                                     rhs=stage[:], start=False, stop=True)

                # nonlinearity streams for layer 1
                e1 = wpool.tile([128, 256], F16, name="e1", tag="e1")
                r1 = wpool.tile([128, 256], F16, name="r1", tag="r1")
                m1 = wpool.tile([128, 256], F16, name="m1", tag="m1")
                for h in range(2):
                    hs = slice(h * 128, (h + 1) * 128)
                    nc.scalar.activation(e1[:, hs], a1[:, hs], AF.Exp)
                    nc.vector.tensor_scalar_max(r1[:, hs], a1[:, hs], 0.0)
                    nc.vector.tensor_scalar_min(m1[:, hs], e1[:, hs], 1.0)

                # L2
                a2 = ppool.tile([128, 256], F32, name="a2", tag="a2")
                for h in range(2):
                    hs = slice(h * 128, (h + 1) * 128)
                    nc.tensor.matmul(a2[:, hs], lhsT=sb["b2e"][:, hs],
                                     rhs=sb["ones2"][:], start=True, stop=False)
                    for kc in range(2):
                        lh = sb["w2p"][:, (kc * 2 + h) * 128:(kc * 2 + h + 1) * 128]
                        ks = slice(kc * 128, (kc + 1) * 128)
                        nc.tensor.matmul(a2[:, hs], lhsT=lh, rhs=r1[:, ks],
                                         start=False, stop=False)
                    for kc in range(2):
                        lh = sb["w2p"][:, (kc * 2 + h) * 128:(kc * 2 + h + 1) * 128]
                        ks = slice(kc * 128, (kc + 1) * 128)
                        nc.tensor.matmul(a2[:, hs], lhsT=lh, rhs=m1[:, ks],
                                         start=False, stop=(kc == 1))
                e2 = wpool.tile([128, 256], F16, name="e2", tag="e2")
                r2 = wpool.tile([128, 256], F16, name="r2", tag="r2")
                m2 = wpool.tile([128, 256], F16, name="m2", tag="m2")
                for h in range(2):
                    hs = slice(h * 128, (h + 1) * 128)
                    nc.scalar.activation(e2[:, hs], a2[:, hs], AF.Exp)
                    nc.vector.tensor_scalar_max(r2[:, hs], a2[:, hs], 0.0)
                    nc.vector.tensor_scalar_min(m2[:, hs], e2[:, hs], 1.0)

                # L3
                a3 = ppool.tile([128, 256], F32, name="a3", tag="a3")
                for h in range(2):
                    hs = slice(h * 128, (h + 1) * 128)
                    nc.tensor.matmul(a3[:, hs], lhsT=sb["b3e"][:, hs],
                                     rhs=sb["ones2"][:], start=True, stop=False)
                    for kc in range(2):
                        lh = sb["w3p"][:, (kc * 2 + h) * 128:(kc * 2 + h + 1) * 128]
                        ks = slice(kc * 128, (kc + 1) * 128)
                        nc.tensor.matmul(a3[:, hs], lhsT=lh, rhs=r2[:, ks],
                                         start=False, stop=False)
                    for kc in range(2):
                        lh = sb["w3p"][:, (kc * 2 + h) * 128:(kc * 2 + h + 1) * 128]
                        ks = slice(kc * 128, (kc + 1) * 128)
                        nc.tensor.matmul(a3[:, hs], lhsT=lh, rhs=m2[:, ks],
                                         start=False, stop=(kc == 1))
                e3 = wpool.tile([128, 256], F16, name="e3", tag="e3")
                r3 = wpool.tile([128, 256], F16, name="r3", tag="r3")
                m3 = wpool.tile([128, 256], F16, name="m3", tag="m3")
                for h in range(2):
                    hs = slice(h * 128, (h + 1) * 128)
                    nc.scalar.activation(e3[:, hs], a3[:, hs], AF.Exp)
                    nc.vector.tensor_scalar_max(r3[:, hs], a3[:, hs], 0.0)
                    nc.vector.tensor_scalar_min(m3[:, hs], e3[:, hs], 1.0)

                # L4: dx = h3 @ W4 + b4eff
                dxp = ppool.tile([8, 128], F32, name="dxp", tag="dxp")
                nc.tensor.matmul(dxp[:], lhsT=sb["b4e"][:],
                                 rhs=sb["ones2"][0:2, :],
                                 start=True, stop=False)
                for kc in range(2):
                    lh = sb["w4p"][:, kc * 8:(kc + 1) * 8]
                    ks = slice(kc * 128, (kc + 1) * 128)
                    nc.tensor.matmul(dxp[:], lhsT=lh, rhs=r3[:, ks],
                                     start=False, stop=False)
                    nc.tensor.matmul(dxp[:], lhsT=lh, rhs=m3[:, ks],
                                     start=False, stop=(kc == 1))

                # Euler into the output ring: x_{i+1} = dt_i * dx + x_i
                if g not in rings:
                    rings[g] = wpool.tile([8, SPG * PB], F32, name="xring",
                                          tag="xring", bufs=2)
                xnext = rings[g][:, s * PB:(s + 1) * PB]
                nc.vector.scalar_tensor_tensor(
                    out=xnext, in0=dxp[:],
                    scalar=sb["dtvec"][:, i:i + 1], in1=xprev,
                    op0=OP.mult, op1=OP.add)
                xprev = xnext
                # flush ring when full (or at the last step)
                if s == SPG - 1 or i == nsteps - 1:
                    nfill = s + 1
                    nc.sync.dma_start(
                        out=xout_d[:, g * SPG + 1:g * SPG + 1 + nfill, :],
                        in_=rings[g][:, :nfill * PB])
                # next step's staging rows (fp16 x)
                if i + 1 < nsteps:
                    nstage = stages[(i + 1) % 3]
                    nc.vector.tensor_copy(out=nstage[0:8, :], in_=xnext)

    nc.compile()
    return nc


_BUILD_CACHE = {}


def _get_compiled(nsteps, ng):
    key = (nsteps, ng)
    if key not in _BUILD_CACHE:
        _BUILD_CACHE[key] = _build(nsteps, ng)
    return _BUILD_CACHE[key]


def _pack_stream(a, ng):
    """[PB, ng*16, 8] -> [128, ng*128] with row s16*8+f, col g*128+j."""
    pb = a.shape[0]
    return (a.transpose(1, 2, 0)
             .reshape(ng, SPG, 8, pb)
             .transpose(1, 2, 0, 3)
             .reshape(128, ng * pb))


def _hilo(v):
    hi = v.astype(np.float16)
    lo = (v - hi.astype(np.float32)).astype(np.float16)
    return np.stack([hi, lo]).reshape(2, -1)


def kernel(t, x, z, event_t, z_jump, W1, b1, W2, b2, W3, b3, W4, b4,
           nsteps=NSTEPS, ng=NG):
    global LAST_RESULTS
    t = np.asarray(t, np.float32); x = np.asarray(x, np.float32)
    z = np.asarray(z, np.float32)
    event_t = np.asarray(event_t, np.float32)
    z_jump = np.asarray(z_jump, np.float32)
    W1 = np.asarray(W1, np.float32); b1 = np.asarray(b1, np.float32)
    W2 = np.asarray(W2, np.float32); b2 = np.asarray(b2, np.float32)
    W3 = np.asarray(W3, np.float32); b3 = np.asarray(b3, np.float32)
    W4 = np.asarray(W4, np.float32); b4 = np.asarray(b4, np.float32)

    nslot = ng * SPG
    tv = t[0, :, 0]                       # identical across batch
    nuse = min(T - 1, nslot)
    dt = np.zeros(nslot, np.float32)
    dt[:nuse] = tv[1:nuse + 1] - tv[:nuse]
    tp = np.zeros(nslot, np.float32)
    tp[:nuse] = tv[:nuse]
    tp[nuse:] = tv[nuse - 1]

    tvec = np.repeat(tp.reshape(ng, SPG).T, 8, axis=0).astype(np.float32)
    dtvec = np.broadcast_to(dt, (8, nslot)).astype(np.float32).copy()

    # shared weight-derived tensors
    W1a, W1b, W1c = W1[0:16], W1[16:32], W1[32:48]
    V = (W1a - W1b).astype(np.float32)
    U = (W1b + W1c).astype(np.float32)
    vux = np.zeros((128, 256), np.float16)
    vux[0:8] = U[:8].astype(np.float16)
    vux[8:24] = V.astype(np.float16)
    vux[24:26] = _hilo(b1).reshape(2, 256)
    uz = U[8:16].astype(np.float16)      # [8, 256]
    uzv = np.zeros((128, 16, 256), np.float16)
    for r in range(16):
        uzv[8 * r:8 * r + 8, r] = uz
    uzv = uzv.reshape(128, 16 * 256)
    w2p = W2.reshape(2, 128, 2, 128).transpose(1, 0, 2, 3).reshape(128, 512)
    w3p = W3.reshape(2, 128, 2, 128).transpose(1, 0, 2, 3).reshape(128, 512)
    w4p = W4.reshape(2, 128, XD).transpose(1, 0, 2).reshape(128, 2 * XD)
    def _hilo_pad(v):
        out = np.zeros((128, v.shape[0]), np.float32)
        out[0:2] = _hilo(v)
        return out
    b2e = _hilo_pad(b2 - W2.sum(0))
    b3e = _hilo_pad(b3 - W3.sum(0))
    b4e = _hilo(b4 - W4.sum(0))
    shared = dict(
        tvec=tvec, dtvec=dtvec, vux=vux.astype(np.float16),
        uzv=uzv,
        w2p=w2p.astype(np.float16), w3p=w3p.astype(np.float16),
        w4p=w4p.astype(np.float16),
        b2e=b2e.astype(np.float16), b3e=b3e.astype(np.float16),
        b4e=b4e.astype(np.float16),
        ones2=np.concatenate([np.ones((2, PB), np.float16),
                              np.zeros((126, PB), np.float16)], axis=0),
    )

    in_maps = []
    for c in range(NCORES):
        bs = slice(c * PB, (c + 1) * PB)
        zc = np.zeros((PB, nslot, ZD), np.float32)
        zc[:, :min(T - 1, nslot)] = z[bs, :min(T - 1, nslot)]
        x0 = x[bs, 0]                     # [PB, 8]
        z0 = z[bs, 0]
        aiext = np.concatenate(
            [x0.T, z0.T, np.ones((2, PB), np.float32)], axis=0)
        m = dict(shared)
        m.update(
            zpack=_pack_stream(zc, ng),
            etb=np.broadcast_to(event_t[bs, 0], (128, PB)).copy().astype(np.float32),
            zjb=np.tile(z_jump[bs].T.astype(np.float16), (SPG, 1)),
            aiext=aiext.astype(np.float16),
            x0f=x0.T.astype(np.float32).copy(),
            x0h=x0.T.astype(np.float16).copy(),
        )
        in_maps.append({k: np.ascontiguousarray(v) for k, v in m.items()})

    nc = _get_compiled(nsteps, ng)
    res = bass_utils.run_bass_kernel_spmd(nc, in_maps,
                                          core_ids=list(range(NCORES)))
    LAST_RESULTS = res

    out = np.zeros((B, T, XD), np.float32)
    n = min(nsteps + 1, T)
    for c in range(NCORES):
        raw = res.results[c]["xout_d"]            # [8, nslot+16, PB]
        traj = raw.transpose(2, 1, 0)             # [PB, nslot+16, 8]
        out[c * PB:(c + 1) * PB, 0] = x[c * PB:(c + 1) * PB, 0]
        out[c * PB:(c + 1) * PB, 1:n] = traj[:, 1:n]
    return out

